# revision 9
# baseline (speedup 1.0000x reference)
"""Causal self-attention (B=4, T=2048, E=2048, H=16) on 8 trn2 NeuronCores.

Tensor-parallel over heads: 2 heads per core. Per-core Bass/Tile kernel:
  qkvT = w_qkvT.T @ xT (bf16 matmuls, f32 PSUM), fused rotate-half RoPE
  (DVE), attention in transposed layout (scoresT = k.T@q so softmax'd
  probs feed the PV matmul directly), causal block skipping with
  narrowed diagonal tiles, softmax without max-subtraction (scores are
  O(5), exp cannot overflow).

  PV+Z fusion (the v2 change): instead of v-stationary PV plus a
  separate ones-matmul for the softmax denominator (which costs a full
  extra pass of pt through the PE), pt 128-query chunks are the
  STATIONARY operand and the moving operand is [v | ones] (129
  columns): out = pt_chunk.T @ [v|1] = [queries, HD+1] with the
  denominator in column HD. Normalization is then a per-partition
  reciprocal + tensor_scalar_mul on DVE, and a PE matmul-with-identity
  transposes the normalized [q, d] chunk back to channel-major [d, q]
  for the A2A scatter. Net PE streaming drops ~9% (the Z pass is gone;
  transposes cost 1/8.5 of it back).

  PSUM budget (8 banks): bps 3 (QKV chains + scores), pv 2 (subtile
  chains, sequential per bank so the whole-bank has_written clear on a
  chain's first matmul never hits a live accumulation), tps 1
  (transposes, single-shot), pps 2 (o_proj drains).

  Everything the PE streams is bf16. w_o is resident in SBUF (64
  KB/partition), its load spread across batch 0's compute. x is bf16 in
  HBM.

  Token resharding via EIGHT bf16 AllToAlls as in v1 (128-token blocks
  interleaved across ranks: block g of a batch -> rank g%8, half g//8;
  half 0 fires after qt==1, half 1 at attention end). o_proj for BOTH
  halves of batch b is interleaved into batch b+1's QKV tiles (tiles
  0,1 carry half 0, tiles 2,3 carry half 1), so after each attention
  phase the PE rolls straight into the next QKV with no A2A wait. The
  last batch's tail runs o_proj(last,0), then the deferred
  o_proj(last-1,1), then o_proj(last,1), which covers the final A2A +
  a_tile load with ~34us of matmul work.

Host-side prep in kernel(): transpose x and cast bf16, permute q/k
weight rows so RoPE becomes rotate-half (scores invariant under a shared
d-permutation), fold the 1/sqrt(d) scale into w_q, precompute cos/sin
tables, shard w_qkv by head (bf16), cast w_o to bf16, pass a 128x128
identity for the PE transposes. Device emits bf16; host upcasts to f32.
"""

import sys

sys.path.insert(0, "/opt/trn_rl_repo")

import ml_dtypes
import numpy as np

B, T, E, H = 4, 2048, 2048, 16
HD = E // H            # 128
NC_ = 8                # cores
HPC = H // NC_         # heads per core
CL = 3 * HPC * HD      # local qkv channels = 768
VOFF = 2 * HPC * 128   # column offset of v channels in wqkvT = 512
BLK = T // NC_         # token block per rank per batch = 256
HB = 128               # tokens per A2A half-block
TT = 512               # token tile
EB = E // 128          # 16 contraction blocks
NBT = T // TT          # 4 token tiles per batch
KB = T // 128          # 16 key blocks per batch
VW = HD + 2            # v_hold inner stride (129 used, padded for align)

_BUILT = None


def _build(b_run=B):
    import concourse.mybir as mybir
    import concourse.tile as tile
    from concourse import bacc

    f32 = mybir.dt.float32
    bf16 = mybir.dt.bfloat16
    ACT = mybir.ActivationFunctionType
    MUL = mybir.AluOpType.mult

    BT = b_run * T

    nc = bacc.Bacc("TRN2", target_bir_lowering=False, debug=False,
                   num_devices=NC_)

    xT = nc.dram_tensor("xT", [E, BT], bf16, kind="ExternalInput")
    wqkvT = nc.dram_tensor("wqkvT", [E, CL], bf16, kind="ExternalInput")
    woT = nc.dram_tensor("woT", [E, E], bf16, kind="ExternalInput")
    cosT = nc.dram_tensor("cosT", [64, T], f32, kind="ExternalInput")
    sinT = nc.dram_tensor("sinT", [64, T], f32, kind="ExternalInput")
    trimask = nc.dram_tensor("trimask", [128, 128], bf16,
                             kind="ExternalInput")
    ident = nc.dram_tensor("ident", [128, 128], bf16, kind="ExternalInput")
    outT = nc.dram_tensor("outT", [E, b_run * BLK], bf16,
                          kind="ExternalOutput")

    xT_r = xT.rearrange("(eh p) t -> p eh t", p=128)
    woT_r = woT.rearrange("(cb p) e -> p cb e", p=128)
    wqkvT_r = wqkvT.rearrange("(eb p) c -> p eb c", p=128)

    with tile.TileContext(nc) as tc:
        with tc.tile_pool(name="consts", bufs=1) as consts, \
             tc.tile_pool(name="dram", bufs=1, space="DRAM") as dram, \
             tc.tile_pool(name="wq", bufs=1) as wq_pool, \
             tc.tile_pool(name="xt", bufs=8) as xt_pool, \
             tc.tile_pool(name="qk", bufs=1) as qk_pool, \
             tc.tile_pool(name="pt", bufs=18) as pt_pool, \
             tc.tile_pool(name="eps", bufs=1) as eps_pool, \
             tc.tile_pool(name="wo", bufs=1) as wo_pool, \
             tc.tile_pool(name="oo", bufs=2) as oo_pool, \
             tc.tile_pool(name="bps", bufs=3, space="PSUM") as bps, \
             tc.tile_pool(name="pvps", bufs=1, space="PSUM") as pvps, \
             tc.tile_pool(name="tps", bufs=1, space="PSUM") as tps_pool, \
             tc.tile_pool(name="pps", bufs=2, space="PSUM") as pps:
            cos_sb = consts.tile([64, T], f32)
            sin_sb = consts.tile([64, T], f32)
            tri_sb = consts.tile([128, 128], bf16)
            id_sb = consts.tile([128, 128], bf16)
            nc.sync.dma_start(out=tri_sb[:], in_=trimask[:])
            nc.sync.dma_start(out=id_sb[:], in_=ident[:])

            agl = [[dram.tile([E, HB], bf16, name=f"agl{b}_{h}")
                    for h in range(2)] for b in range(b_run)]
            agf = [[dram.tile([E, HB], bf16, name=f"agf{b}_{h}")
                    for h in range(2)] for b in range(b_run)]

            xcache = {}

            def load_xtile(b, tt, split=False):
                if (b, tt) in xcache:
                    return xcache.pop((b, tt))
                t0 = b * T + tt * TT
                xc = []
                for h in range(4):
                    xch = xt_pool.tile([128, EB // 4, TT], bf16, tag="xt",
                                       name="xch")
                    if split:
                        # per-e-block DMAs so each matmul only waits for
                        # its own 128x512 slice during the cold start
                        for e in range(4):
                            nc.sync.dma_start(
                                out=xch[:, e, :],
                                in_=xT_r[:, h * 4 + e, t0:t0 + TT])
                    else:
                        nc.sync.dma_start(
                            out=xch[:],
                            in_=xT_r[:, h * 4:(h + 1) * 4, t0:t0 + TT])
                    xc.append(xch)
                return xc

            # startup: interleave w_qkv blocks, the first x tile and the
            # first cos/sin slices so the first QKV chain starts ASAP.
            w_sb = wq_pool.tile([128, EB, CL], bf16)

            def _w(e):
                nc.gpsimd.dma_start(out=w_sb[:, e, :], in_=wqkvT_r[:, e, :])

            def _cs(tt):
                sl = slice(tt * TT, (tt + 1) * TT)
                nc.sync.dma_start(out=cos_sb[:, sl], in_=cosT[:, sl])
                nc.sync.dma_start(out=sin_sb[:, sl], in_=sinT[:, sl])

            _w(0)
            x00 = load_xtile(0, 0, split=True)
            _cs(0)
            for e in range(1, EB):
                _w(e)
            for tt in range(1, NBT):
                _cs(tt)
            xcache[(0, 0)] = x00
            xcache[(0, 1)] = load_xtile(0, 1)

            # w_o resident in SBUF for the whole kernel; its 8 MB load is
            # trickled out on the gpsimd queue (so the sync queue stays
            # clear for x tiles) across batch 0's compute, keeping the
            # burst below the activity power-throttle.
            wo_sb = wo_pool.tile([128, EB, E], bf16)
            wo_cb_iter = iter(range(EB))

            def wo_feed(n):
                for _ in range(n):
                    cb = next(wo_cb_iter, None)
                    if cb is None:
                        return
                    nc.gpsimd.dma_start(out=wo_sb[:, cb, :],
                                        in_=woT_r[:, cb, :])

            wo_feed(4)

            def wv(e, cs):
                return w_sb[:, e, cs]

            atiles = {}

            def prefetch_atile(b, h):
                a_tile = oo_pool.tile([128, EB, HB], bf16, tag="at",
                                      name="a_tile", bufs=3)
                nc.gpsimd.dma_start(
                    out=a_tile[:],
                    in_=agf[b][h].rearrange("(cb p) t -> p cb t", p=128))
                atiles[(b, h)] = a_tile

            def emit_oproj(b, h, ebs, dve_drain=False):
                # two ebs share one PSUM tile / drain copy so the
                # per-chunk PSUM-drain latency amortizes over 2x the
                # matmul work. Chunks interleaved into attention drain on
                # DVE (scalar is saturated by softmax exps there); chunks
                # in the QKV phase drain on scalar (DVE does RoPE there).
                a_tile = atiles[(b, h)]
                ebs = list(ebs)
                for i in range(0, len(ebs), 2):
                    pair = ebs[i:i + 2]
                    pso = pps.tile([128, len(pair) * HB], f32, tag="pso",
                                   name="pso")
                    for pi, eb in enumerate(pair):
                        for cb in range(EB):
                            nc.tensor.matmul(
                                pso[:, pi * HB:(pi + 1) * HB],
                                wo_sb[:, cb, eb * 128:(eb + 1) * 128],
                                a_tile[:, cb, :],
                                start=(cb == 0), stop=(cb == EB - 1))
                    ot = oo_pool.tile([128, len(pair) * HB], bf16, tag="ot",
                                      name="ot", bufs=2)
                    if dve_drain:
                        nc.vector.tensor_copy(ot[:], pso[:])
                    else:
                        nc.scalar.activation(ot[:], pso[:], ACT.Copy)
                    for pi, eb in enumerate(pair):
                        nc.sync.dma_start(
                            out=outT[eb * 128:(eb + 1) * 128,
                                     b * BLK + h * HB:b * BLK + (h + 1) * HB],
                            in_=ot[:, pi * HB:(pi + 1) * HB])

            # Global FIFO of pending o_proj eb-units. Attention qts pop
            # [2,3,4,7] units (filling the exp-bound PE slack), QKV tiles
            # pop 4, the tail pops the rest. A unit (b', h') only pops
            # once its a_tile is safely resident: half 0 from attn(b')
            # qt3 on, half 1 from QKV(b'+1) tile 1 on.
            oq = []

            def oq_ready(u, phase, b, idx):
                ub, uh, _ = u
                if uh == 0:
                    if phase == 'attn':
                        return ub < b or (ub == b and idx == 3)
                    return ub < b
                if phase == 'attn':
                    return ub < b
                return ub < b - 1 or (ub == b - 1 and idx >= 1)

            def oq_pop(k, phase, b, idx, dve_drain=False):
                while k > 0 and oq and oq_ready(oq[0], phase, b, idx):
                    ub, uh, _ = oq[0]
                    ebs = []
                    while (k > 0 and oq and oq[0][0] == ub
                           and oq[0][1] == uh):
                        ebs.append(oq.pop(0)[2])
                        k -= 1
                    emit_oproj(ub, uh, ebs, dve_drain=dve_drain)

            for b in range(b_run):
                q_sb = [qk_pool.tile([HD, T], bf16, tag=f"q{j}",
                                     name=f"q_sb{j}") for j in range(HPC)]
                k_sb = [qk_pool.tile([HD, T], bf16, tag=f"k{j}",
                                     name=f"k_sb{j}") for j in range(HPC)]
                # v in token-major layout with a padded ones column per
                # head: v_hold[:, kb, j, 0:HD]=v, [..., HD]=1.0, so the
                # PV moving operand [v | 1] is one contiguous 129-col AP
                v_hold = qk_pool.tile([128, KB, HPC, VW], bf16, tag="vh",
                                      name="v_hold")
                nc.vector.memset(v_hold[:, :, :, HD], 1.0)

                # ---- QKV projection for this batch; previous batch's
                # o_proj (both halves) interleaved at tile boundaries ----
                for tt in range(NBT):
                    xc = load_xtile(b, tt)

                    def xv(e, ts=slice(None)):
                        return xc[e // 4][:, e % 4, ts]

                    # q/k channels: c-blocks [q0,k0,q1,k1]
                    for c in range(2 * HPC):
                        j, is_k = c // 2, c % 2
                        ps = bps.tile([128, TT], f32, tag="big", name="ps_qk")
                        for e in range(EB):
                            nc.tensor.matmul(
                                ps[:], wv(e, slice(c * 128, (c + 1) * 128)),
                                xv(e), start=(e == 0), stop=(e == EB - 1))
                        # rotate-half rope out of PSUM
                        cs = cos_sb[:, tt * TT:(tt + 1) * TT]
                        sn = sin_sb[:, tt * TT:(tt + 1) * TT]
                        t1 = eps_pool.tile([128, TT], f32, tag="t1",
                                           name="t1", bufs=2)
                        t2 = eps_pool.tile([128, TT], f32, tag="t2",
                                           name="t2", bufs=2)
                        nc.vector.tensor_mul(t1[0:64, :], ps[0:64, :], cs)
                        nc.vector.tensor_mul(t1[64:128, :], ps[64:128, :], cs)
                        nc.vector.scalar_tensor_tensor(
                            t2[0:64, :], ps[64:128, :], -1.0, sn, MUL, MUL)
                        nc.vector.tensor_mul(t2[64:128, :], ps[0:64, :], sn)
                        dst = (k_sb if is_k else q_sb)[j]
                        nc.vector.tensor_add(
                            dst[:, tt * TT:(tt + 1) * TT], t1[:], t2[:])
                    # v channels, natural (t, d) layout, straight to SBUF
                    for tb in range(TT // 128):
                        psv = bps.tile([128, HPC * HD], f32, tag="big",
                                       name="psv")
                        for e in range(EB):
                            nc.tensor.matmul(
                                psv[:], xv(e, slice(tb * 128, (tb + 1) * 128)),
                                wv(e, slice(VOFF, CL)),
                                start=(e == 0), stop=(e == EB - 1))
                        kb = tt * (TT // 128) + tb
                        for j in range(HPC):
                            nc.scalar.activation(
                                v_hold[:, kb, j, 0:HD],
                                psv[:, j * HD:(j + 1) * HD], ACT.Copy)
                    if b == 0:
                        wo_feed(3)
                    oq_pop(4, 'qkv', b, tt)

                # prefetch next batch's first x tile during attention
                if b + 1 < b_run:
                    xcache[(b + 1, 0)] = load_xtile(b + 1, 0)

                # ---- attention ----
                # per (head, qt): scores chain (k stationary, q moving,
                # N=512), then four subtile PV chains with pt 128-query
                # chunks stationary and [v|1] moving. The epilogue
                # (reciprocal, normalize, transpose, scatter) of each
                # (head, qt) is deferred past the next scores chain so
                # its DVE/ACT latency hides under PE work.
                pend = None

                def flush_epi():
                    nonlocal pend
                    if pend is None:
                        return
                    j, qt, pts_l, pv_a, pv_b = pend
                    pend = None
                    zr = eps_pool.tile([128, 4], f32, tag="zr", name="zr",
                                       bufs=2)
                    ao = eps_pool.tile([128, 4, 128], bf16, tag="ao",
                                       name="ao", bufs=2)
                    pst = tps_pool.tile([128, 4, 128], f32, tag="pst",
                                        name="pst")
                    aot = eps_pool.tile([128, 4, 128], bf16, tag="aot",
                                        name="aot", bufs=2)
                    for s in range(4):
                        bank, slot = (pv_a, s) if s < 2 else (pv_b, s - 2)
                        nc.vector.reciprocal(zr[:, s:s + 1],
                                             bank[:, slot, HD:HD + 1])
                        nc.vector.tensor_scalar_mul(
                            ao[:, s, :], bank[:, slot, 0:HD], zr[:, s:s + 1])
                        nc.tensor.matmul(pst[:, s, :], ao[:, s, :], id_sb[:],
                                         start=True, stop=True)
                    nc.vector.tensor_copy(aot[:], pst[:])
                    for s in range(4):
                        g = qt * 4 + s
                        h, rr = g // 8, g % 8
                        nc.sync.dma_start(
                            out=agl[b][h][rr * (E // NC_) + j * HD:
                                          rr * (E // NC_) + (j + 1) * HD, :],
                            in_=aot[:, s, :])

                for qt in range(NBT):
                    nkt = (qt + 1) * (TT // 128)
                    for j in range(HPC):
                        # scores + exp for all key blocks of this qt
                        pts_l = []
                        for kt in range(nkt):
                            m = kt - qt * (TT // 128)
                            lo = max(m, 0) * 128
                            ps_s = bps.tile([128, TT], f32, tag="big",
                                            name="ps_s")
                            nc.tensor.matmul(
                                ps_s[:, lo:],
                                k_sb[j][:, kt * 128:(kt + 1) * 128],
                                q_sb[j][:, qt * TT + lo:(qt + 1) * TT],
                                start=True, stop=True)
                            pt = pt_pool.tile([128, TT], bf16, tag="pt",
                                              name="pt")
                            nc.scalar.activation(
                                pt[:, lo:], ps_s[:, lo:], ACT.Exp)
                            if m >= 0:
                                nc.vector.tensor_mul(
                                    pt[:, lo:lo + 128],
                                    pt[:, lo:lo + 128], tri_sb[:])
                            pts_l.append(pt)
                        # o_proj fill for the exp-bound slack; then the
                        # previous (head, qt)'s epilogue, whose DVE/PE
                        # work runs under the scores chain just emitted
                        oq_pop(((2, 3, 4, 7)[qt] + 1 - j) // 2,
                               'attn', b, qt, dve_drain=True)
                        flush_epi()
                        # PV+Z: subtile-outer accumulation chains; two
                        # subtiles per PSUM bank, chains strictly
                        # sequential within a bank
                        pv_a = pvps.tile([128, 2, VW], f32, tag="pva",
                                         name="pv_a")
                        pv_b = pvps.tile([128, 2, VW], f32, tag="pvb",
                                         name="pv_b")
                        for s in range(4):
                            bank, slot = (pv_a, s) if s < 2 else (pv_b, s - 2)
                            last = qt * 4 + s
                            for kt in range(last + 1):
                                nc.tensor.matmul(
                                    bank[:, slot, 0:HD + 1],
                                    pts_l[kt][:, s * 128:(s + 1) * 128],
                                    v_hold[:, kt, j, 0:HD + 1],
                                    start=(kt == 0), stop=(kt == last))
                        pend = (j, qt, pts_l, pv_a, pv_b)
                    if qt == 1:
                        flush_epi()
                        nc.gpsimd.collective_compute(
                            "AllToAll", mybir.AluOpType.bypass,
                            replica_groups=[list(range(NC_))],
                            ins=[agl[b][0][:]], outs=[agf[b][0][:]])
                        prefetch_atile(b, 0)
                        oq.extend((b, 0, eb) for eb in range(EB))

                # fire half 1's A2A; o_proj work (the FIFO backlog)
                # covers it
                flush_epi()
                nc.gpsimd.collective_compute(
                    "AllToAll", mybir.AluOpType.bypass,
                    replica_groups=[list(range(NC_))],
                    ins=[agl[b][1][:]], outs=[agf[b][1][:]])
                prefetch_atile(b, 1)
                oq.extend((b, 1, eb) for eb in range(EB))

            # tail: drain the o_proj FIFO (the last batch's half 0
            # remainder covers the final A2A + a_tile load)
            while oq:
                ub, uh, _ = oq[0]
                ebs = []
                while oq and oq[0][0] == ub and oq[0][1] == uh:
                    ebs.append(oq.pop(0)[2])
                emit_oproj(ub, uh, ebs)
    nc.compile()
    return nc


def _prep_inputs(x, freqs, w_qkv, w_o, b_run=B):
    bf16 = ml_dtypes.bfloat16
    xf = np.ascontiguousarray(x, dtype=np.float32).reshape(b_run * T, E)
    xT = np.ascontiguousarray(xf.T.astype(bf16))

    wq = w_qkv[0:E].reshape(H, HD, E)
    wk = w_qkv[E:2 * E].reshape(H, HD, E)
    wvv = w_qkv[2 * E:3 * E].reshape(H, HD, E)
    perm = np.concatenate([np.arange(0, HD, 2), np.arange(1, HD, 2)])
    scale = np.float32(1.0 / np.sqrt(HD))
    wq_p = wq[:, perm, :] * scale
    wk_p = wk[:, perm, :]

    cos = np.cos(freqs.astype(np.float32))
    sin = np.sin(freqs.astype(np.float32))
    cosT = np.ascontiguousarray(cos.T)
    sinT = np.ascontiguousarray(sin.T)
    tri = (np.arange(128)[:, None] <= np.arange(128)[None, :]).astype(bf16)
    ident = np.eye(128, dtype=np.float32).astype(bf16)
    w_oT = np.ascontiguousarray(w_o.T.astype(bf16))

    in_maps = []
    for r in range(NC_):
        blocks = []
        for j in range(HPC):
            h = r * HPC + j
            blocks += [wq_p[h].T, wk_p[h].T]
        blocks += [wvv[r * HPC + j].T for j in range(HPC)]
        wqkvT_loc = np.ascontiguousarray(
            np.concatenate(blocks, axis=1).astype(bf16))
        in_maps.append({
            "xT": xT,
            "wqkvT": wqkvT_loc,
            "woT": w_oT,
            "cosT": cosT,
            "sinT": sinT,
            "trimask": tri,
            "ident": ident,
        })
    return in_maps


def kernel(x, freqs, w_qkv, w_o, _trace=False, _b_run=B):
    global _BUILT
    from concourse.bass_utils import run_bass_kernel_spmd

    if _BUILT is None or _BUILT[1] != _b_run:
        _BUILT = (_build(_b_run), _b_run)
    nc = _BUILT[0]

    in_maps = _prep_inputs(np.asarray(x), np.asarray(freqs),
                           np.asarray(w_qkv), np.asarray(w_o), _b_run)
    res = run_bass_kernel_spmd(nc, in_maps, core_ids=list(range(NC_)),
                               trace=_trace)
    # core r owns token block g of each batch iff g%8 == r, stored in its
    # outT at columns [b*BLK + (g//8)*HB, ...)
    out = np.empty((E, _b_run * T), np.float32)
    for r in range(NC_):
        o = res.results[r]["outT"]
        for b in range(_b_run):
            for h in range(2):
                g = 8 * h + r
                out[:, b * T + g * HB:b * T + (g + 1) * HB] = \
                    o[:, b * BLK + h * HB:b * BLK + (h + 1) * HB] \
                    .astype(np.float32)
    out = np.ascontiguousarray(out.T).reshape(_b_run, T, E)
    if _trace:
        kernel.last_results = res
    return out.astype(np.float32, copy=False)
